# revision 23
# baseline (speedup 1.0000x reference)
"""Trainium2 Bass kernel for nn_Encoder (KAN-style piecewise-linear MLP encoder).

Math: each adaptive piecewise-linear layer (P=3 knots on [-1,1]) collapses to
    out = u @ A + v @ C + bias,   u = clip(x,-1,1), v = clip(x,0,1)
with A = V1-V0, C = V0+V2-2*V1, bias = colsum(V1)  (hat basis sums to 1).
ELU never needs materializing: the next layer only consumes
    v' = clip(elu(h),0,1) = clip(h,0,1)
    u' = clip(elu(h),-1,1) + 1 = v' + exp(min(h,0))
and the +1 shift is folded into the next layer's bias (bias -= colsum(A_rows)).

Sharding: pure data-parallel, batch 16384 -> 8 x 2048. Activations are kept
feature-major ([feat, batch]) on chip so every matmul contracts over the
partition dim with no transposes; the host transposes the zs/action shards.
LayerNorm stats (feature = partition axis) are computed with ones-matmuls on
the PE and broadcast back with K=1 rank-1 matmuls. The last layer is computed
batch-major (activations stationary) so the output DMAs straight from PSUM in
the required [batch, 512] layout. Matmuls run as float32r (full PE rate).
"""

import sys

sys.path.insert(0, "/opt/trn_rl_repo")

import numpy as np

import concourse.bass as bass  # noqa: E402
import concourse.tile as tile  # noqa: E402
from concourse import bacc, mybir  # noqa: E402
from concourse.bass_utils import run_bass_kernel_spmd  # noqa: E402

F32 = mybir.dt.float32
F32R = mybir.dt.float32r
AF = mybir.ActivationFunctionType
OP = mybir.AluOpType

NCORES = 8
B_LOC = 2048          # batch rows per core
BC = 512              # batch columns per chunk (psum free dim)
NB = B_LOC // BC      # 4 batch chunks
P = 128
LN_EPS = 1e-5


def build_module(trace_label=None):
    nc = bacc.Bacc("TRN2", target_bir_lowering=False, debug=False,
                   enable_asserts=False, num_devices=NCORES)

    dram = {}
    def din(name, shape, dt=F32):
        dram[name] = nc.dram_tensor(name, list(shape), dt, kind="ExternalInput").ap()
        return dram[name]

    zsT = din("zsT", (512, B_LOC))
    actT = din("actT", (8, B_LOC))
    wza = din("wza", (8, 2, 256), F32R)
    w1 = din("w1", (1536, 512), F32R)
    w2 = din("w2", (1024, 512), F32R)
    w3 = din("w3", (1024, 512), F32R)
    bza_p = din("bza_p", (128, 2))
    b1_p = din("b1_p", (128, 4))
    b2_p = din("b2_p", (128, 4))
    b3_r = din("b3_r", (1, 512), F32R)
    ones_c = din("ones_c", (1, 128), F32R)
    oinv_m = din("oinv_m", (128, 1), F32R)
    out = nc.dram_tensor("out", [B_LOC, 512], F32, kind="ExternalOutput").ap()

    with tile.TileContext(nc) as tc:
        with (
            tc.tile_pool(name="wpool", bufs=1) as wp,
            tc.tile_pool(name="io", bufs=3) as io,
            tc.tile_pool(name="uv", bufs=1) as uv,
            tc.tile_pool(name="eph", bufs=2) as eph,
            tc.tile_pool(name="zcbp", bufs=1) as zcbp,
            tc.tile_pool(name="rows", bufs=1) as rows,
            tc.tile_pool(name="psz", bufs=2, space="PSUM") as psz,
            tc.tile_pool(name="psmu", bufs=1, space="PSUM") as psmu,
            tc.tile_pool(name="pssq", bufs=1, space="PSUM") as pssq,
            tc.tile_pool(name="psbc", bufs=2, space="PSUM") as psbc,
            tc.tile_pool(name="pso", bufs=2, space="PSUM") as pso,
        ):
            # ---- persistent weights / constants ----
            w1_sb = wp.tile([P, 12, 512], F32R)
            nc.sync.dma_start(w1_sb[:], w1.rearrange("(c p) o -> p c o", p=P))
            w2_sb = wp.tile([P, 8, 512], F32R)
            nc.sync.dma_start(w2_sb[:], w2.rearrange("(c p) o -> p c o", p=P))
            w3_sb = wp.tile([P, 8, 512], F32R)
            nc.sync.dma_start(w3_sb[:], w3.rearrange("(c p) o -> p c o", p=P))
            wza_sb = wp.tile([8, 2, 256], F32R)
            nc.sync.dma_start(wza_sb[:], wza[:, :, :])
            bza_sb = wp.tile([P, 2], F32)
            nc.sync.dma_start(bza_sb[:], bza_p[:, :])
            b1_sb = wp.tile([P, 4], F32)
            nc.sync.dma_start(b1_sb[:], b1_p[:, :])
            b2_sb = wp.tile([P, 4], F32)
            nc.sync.dma_start(b2_sb[:], b2_p[:, :])
            b3_sb = wp.tile([1, 512], F32R)
            nc.sync.dma_start(b3_sb[:], b3_r[:, :])
            ones_col = wp.tile([1, 128], F32R)
            nc.sync.dma_start(ones_col[:], ones_c[:, :])
            oinv_mcol = wp.tile([P, 1], F32R)   # 1/512 -> stats matmuls yield means
            nc.sync.dma_start(oinv_mcol[:], oinv_m[:, :])
            eps_sb = wp.tile([1, 1], F32)
            nc.vector.memset(eps_sb[:], LN_EPS)

            actT_sb = wp.tile([8, B_LOC], F32)
            nc.sync.dma_start(actT_sb[:], actT[:, :])

            def hidden_apply(zps_o, b_sb_col, negm_fn_state, o, layer_tag, b,
                             up_dst, v_dst):
                """zps -> zcb(SBUF,+bias); returns zcb for stats."""
                zcb = eph.tile([P, BC], F32, tag=f"zcb{o}")
                nc.scalar.activation(zcb[:], zps_o[:], AF.Identity,
                                     bias=b_sb_col)
                return zcb

            for b in range(NB):
                bs = slice(b * BC, (b + 1) * BC)

                # ======== l0: za = (pre-elu) APL(action) ========
                u_act = eph.tile([8, BC], F32R, tag="u_act")
                nc.gpsimd.tensor_scalar(u_act[:], actT_sb[:, bs],
                                        -1.0, 1.0, OP.max, OP.min)
                v_act = eph.tile([8, BC], F32R, tag="v_act")
                nc.gpsimd.tensor_scalar(v_act[:], actT_sb[:, bs],
                                        0.0, 1.0, OP.max, OP.min)
                up_za = uv.tile([P, 2, BC], F32R, tag="up_za")
                v_za = uv.tile([P, 2, BC], F32R, tag="v_za")
                for o in range(2):
                    zps = psz.tile([P, BC], F32, tag="z")
                    nc.tensor.matmul(zps[:], (wza_sb[:, 0, bass.ts(o, 128)]),
                                     (u_act[:]), start=True, stop=False)
                    nc.tensor.matmul(zps[:], (wza_sb[:, 1, bass.ts(o, 128)]),
                                     (v_act[:]), start=False, stop=True)
                    zb = eph.tile([P, BC], F32, tag="zb_za")
                    nc.scalar.activation(zb[:], zps[:], AF.Identity,
                                         bias=bza_sb[:, o:o + 1])
                    nc.vector.tensor_scalar(v_za[:, o, :], zb[:], 0.0, 1.0,
                                            OP.max, OP.min)
                    nmin = eph.tile([P, BC], F32, tag="nmin_za")
                    nc.vector.tensor_scalar(nmin[:], zb[:], 0.0, None, OP.min)
                    ex = eph.tile([P, BC], F32, tag="ex_za")
                    nc.scalar.activation(ex[:], nmin[:], AF.Exp)
                    nc.vector.tensor_add(up_za[:, o, :], v_za[:, o, :], ex[:])

                # ======== zs clips for l1 ========
                u_zs = uv.tile([P, 4, BC], F32R, tag="u_zs")
                v_zs = uv.tile([P, 4, BC], F32R, tag="v_zs")
                for c in range(4):
                    zsraw = io.tile([P, BC], F32, tag="zsraw")
                    nc.sync.dma_start(zsraw[:], zsT[bass.ts(c, 128), bs])
                    nc.gpsimd.tensor_scalar(u_zs[:, c, :], zsraw[:],
                                            -1.0, 1.0, OP.max, OP.min)
                    nc.gpsimd.tensor_scalar(v_zs[:, c, :], zsraw[:],
                                            0.0, 1.0, OP.max, OP.min)

                # ======== hidden layers l1, l2 ========
                def hidden_layer(KC, w_sb, b_sb, rhs_fn, up_dst, v_dst, ltag):
                    zcbs = []
                    for o in range(4):
                        zps = psz.tile([P, BC], F32, tag="z")
                        for k in range(KC):
                            nc.tensor.matmul(zps[:],
                                             (w_sb[:, k, bass.ts(o, 128)]),
                                             (rhs_fn(k)),
                                             start=(k == 0), stop=(k == KC - 1))
                        zcb = zcbp.tile([P, BC], F32R, tag=f"zcb{o}")
                        nc.scalar.activation(zcb[:], zps[:], AF.Identity,
                                             bias=b_sb[:, o:o + 1])
                        zcbs.append(zcb)
                    # stats: mean and mean-square via ones-matmuls (M=1)
                    mu_ps = psmu.tile([1, BC], F32, tag="mu")
                    for o in range(4):
                        nc.tensor.matmul(mu_ps[:], (oinv_mcol[:]),
                                         (zcbs[o][:]),
                                         start=(o == 0), stop=(o == 3))
                    sq_ps = pssq.tile([1, BC], F32, tag="sq")
                    for o in range(4):
                        zsq = eph.tile([P, BC], F32R, tag="zsq")
                        nc.scalar.activation(zsq[:], zcbs[o][:], AF.Square)
                        nc.tensor.matmul(sq_ps[:], (oinv_mcol[:]),
                                         (zsq[:]),
                                         start=(o == 0), stop=(o == 3))
                    negm = rows.tile([1, BC], F32R, tag="negm")
                    nc.vector.tensor_scalar(negm[:], mu_ps[:], -1.0, None,
                                            OP.mult)
                    m2 = rows.tile([1, BC], F32, tag="m2")
                    nc.scalar.activation(m2[:], negm[:], AF.Square)
                    var = rows.tile([1, BC], F32, tag="var")
                    nc.vector.scalar_tensor_tensor(var[:], sq_ps[:], 1.0,
                                                   m2[:], OP.mult, OP.subtract)
                    std = rows.tile([1, BC], F32, tag="std")
                    nc.scalar.activation(std[:], var[:], AF.Sqrt,
                                         bias=eps_sb[:])
                    s_row = rows.tile([1, BC], F32R, tag="s_row")
                    with nc.allow_low_precision(reason="rstd rounds to f32r for the PE broadcast"):
                        nc.vector.reciprocal(s_row[:], std[:])
                    # broadcast -m and s across partitions via K=1 matmuls
                    mb_ps = psbc.tile([P, BC], F32, tag="bc")
                    nc.tensor.matmul(mb_ps[:], (ones_col[:]), (negm[:]),
                                     start=True, stop=True)
                    sb_ps = psbc.tile([P, BC], F32, tag="bc")
                    nc.tensor.matmul(sb_ps[:], (ones_col[:]), (s_row[:]),
                                     start=True, stop=True)
                    for o in range(4):
                        tmp = eph.tile([P, BC], F32, tag="tmp")
                        nc.vector.tensor_add(tmp[:], zcbs[o][:], mb_ps[:])
                        h = eph.tile([P, BC], F32, tag="h")
                        nc.vector.tensor_mul(h[:], tmp[:], sb_ps[:])
                        nc.gpsimd.tensor_scalar(v_dst[:, o, :], h[:], 0.0, 1.0,
                                                OP.max, OP.min)
                        nmin = eph.tile([P, BC], F32, tag="nmin")
                        nc.gpsimd.tensor_scalar(nmin[:], h[:], 0.0, None,
                                                OP.min)
                        ex = eph.tile([P, BC], F32, tag="ex")
                        nc.scalar.activation(ex[:], nmin[:], AF.Exp)
                        nc.vector.tensor_add(up_dst[:, o, :], v_dst[:, o, :],
                                             ex[:])

                def rhs1(k):
                    if k < 4:
                        return u_zs[:, k, :]
                    if k < 8:
                        return v_zs[:, k - 4, :]
                    if k < 10:
                        return up_za[:, k - 8, :]
                    return v_za[:, k - 10, :]

                up1 = uv.tile([P, 4, BC], F32R, tag="up1")
                v1 = uv.tile([P, 4, BC], F32R, tag="v1")
                hidden_layer(12, w1_sb, b1_sb, rhs1, up1, v1, "l1")

                def rhs2(k):
                    return up1[:, k, :] if k < 4 else v1[:, k - 4, :]

                up2 = uv.tile([P, 4, BC], F32R, tag="up2")
                v2 = uv.tile([P, 4, BC], F32R, tag="v2")
                hidden_layer(8, w2_sb, b2_sb, rhs2, up2, v2, "l2")

                # ======== l3: batch-major out ========
                for q in range(4):
                    qs = bass.ts(q, 128)
                    ops = pso.tile([P, 512], F32, tag="o3")
                    for k in range(8):
                        lhsT = up2[:, k, qs] if k < 4 else v2[:, k - 4, qs]
                        nc.tensor.matmul(ops[:], (lhsT), (w3_sb[:, k, :]),
                                         start=(k == 0), stop=False)
                    nc.tensor.matmul(ops[:], (ones_col[:]), (b3_sb[:]),
                                     start=False, stop=True)
                    osb = eph.tile([P, 512], F32, tag="osb")
                    nc.scalar.activation(osb[:], ops[:], AF.Copy)
                    nc.sync.dma_start(out[b * BC + q * 128:
                                          b * BC + (q + 1) * 128, :], osb[:])

    nc.compile()
    return nc


def fold_weights(W_za, W1, W2, W3):
    def fold(vals):
        V = vals.astype(np.float64)
        A = V[:, :, 1] - V[:, :, 0]
        C = V[:, :, 0] + V[:, :, 2] - 2.0 * V[:, :, 1]
        b = V[:, :, 1].sum(axis=0)
        return A, C, b

    A0, C0, b0 = fold(W_za)
    A1, C1, b1 = fold(W1)
    A2, C2, b2 = fold(W2)
    A3, C3, b3 = fold(W3)

    wza = np.stack([A0, C0], axis=1)                             # [8, 2, 256]
    w1 = np.concatenate([A1[:512], C1[:512], A1[512:], C1[512:]], axis=0)
    w2 = np.concatenate([A2, C2], axis=0)                        # [1024, 512]
    w3 = np.concatenate([A3, C3], axis=0)                        # [1024, 512]
    b1e = b1 - A1[512:].sum(axis=0)      # za u' carries +1 shift
    b2e = b2 - A2.sum(axis=0)
    b3e = b3 - A3.sum(axis=0)

    f = np.float32
    return {
        "wza": np.ascontiguousarray(wza, f),
        "w1": np.ascontiguousarray(w1, f),
        "w2": np.ascontiguousarray(w2, f),
        "w3": np.ascontiguousarray(w3, f),
        "bza_p": np.ascontiguousarray(b0.reshape(2, 128).T, f),
        "b1_p": np.ascontiguousarray(b1e.reshape(4, 128).T, f),
        "b2_p": np.ascontiguousarray(b2e.reshape(4, 128).T, f),
        "b3_r": np.ascontiguousarray(b3e.reshape(1, 512), f),
        "ones_c": np.ones((1, 128), f),
        "oinv_m": np.full((128, 1), 1.0 / 512.0, f),
    }


_NC_CACHE = {}


def get_module():
    if "nc" not in _NC_CACHE:
        _NC_CACHE["nc"] = build_module()
    return _NC_CACHE["nc"]


def make_in_maps(zs, action, W_za, W1, W2, W3):
    wmap = fold_weights(np.asarray(W_za), np.asarray(W1), np.asarray(W2),
                        np.asarray(W3))
    in_maps = []
    for c in range(NCORES):
        sl = slice(c * B_LOC, (c + 1) * B_LOC)
        m = dict(wmap)
        m["zsT"] = np.ascontiguousarray(np.asarray(zs)[sl].T, np.float32)
        m["actT"] = np.ascontiguousarray(np.asarray(action)[sl].T, np.float32)
        in_maps.append(m)
    return in_maps


def kernel(zs, action, W_za, W1, W2, W3, _trace=False, _tmpdir=None):
    nc = get_module()
    in_maps = make_in_maps(zs, action, W_za, W1, W2, W3)
    res = run_bass_kernel_spmd(nc, in_maps, core_ids=list(range(NCORES)),
                               trace=_trace, tmpdir=_tmpdir)
    out = np.concatenate([res.results[c]["out"] for c in range(NCORES)],
                         axis=0).astype(np.float32)
    if _trace:
        kernel.last_exec_time_ns = res.exec_time_ns
        kernel.last_results = res
    return out


# revision 25
# speedup vs baseline: 104.1647x; 104.1647x over previous
"""Trainium2 Bass kernel for nn_Encoder (KAN-style piecewise-linear MLP encoder).

Math: each adaptive piecewise-linear layer (P=3 knots on [-1,1]) collapses to
    out = u @ A + v @ C + bias,   u = clip(x,-1,1), v = clip(x,0,1)
with A = V1-V0, C = V0+V2-2*V1, bias = colsum(V1)  (hat basis sums to 1).
ELU never needs materializing: the next layer only consumes
    v' = clip(elu(h),0,1) = clip(h,0,1)
    u' = clip(elu(h),-1,1) + 1 = v' + exp(min(h,0))
and the +1 shift is folded into the next layer's bias (bias -= colsum(A_rows)).

Sharding: pure data-parallel, batch 16384 -> 8 x 2048. Activations are kept
feature-major ([feat, batch]) on chip so every matmul contracts over the
partition dim with no transposes; the host transposes the zs/action shards.
LayerNorm stats (feature = partition axis) are computed with ones-matmuls on
the PE and broadcast back with K=1 rank-1 matmuls. The last layer is computed
batch-major (activations stationary) so the output DMAs from SBUF in the
required [batch, 512] layout. Matmuls run as float32r (full PE rate).

n_reps>1 wraps the whole computation in a hardware For-loop; used only by the
local timing harness to measure per-iteration device time by wall-clock slope.
"""

import contextlib
import sys

sys.path.insert(0, "/opt/trn_rl_repo")

import numpy as np

import concourse.bass as bass  # noqa: E402
import concourse.tile as tile  # noqa: E402
from concourse import bacc, mybir  # noqa: E402
from concourse.bass_utils import run_bass_kernel_spmd  # noqa: E402

F32 = mybir.dt.float32
F32R = mybir.dt.float32r
AF = mybir.ActivationFunctionType
OP = mybir.AluOpType

NCORES = 8
B_LOC = 2048          # batch rows per core
BC = 512              # batch columns per chunk (psum free dim)
NB = B_LOC // BC      # 4 batch chunks
P = 128
LN_EPS = 1e-5


def build_module(n_reps=1):
    nc = bacc.Bacc("TRN2", target_bir_lowering=False, debug=False,
                   enable_asserts=False, num_devices=NCORES)

    def din(name, shape, dt=F32):
        return nc.dram_tensor(name, list(shape), dt, kind="ExternalInput").ap()

    zsT = din("zsT", (512, B_LOC))
    actT = din("actT", (8, B_LOC))
    wza = din("wza", (8, 2, 256), F32R)
    w1 = din("w1", (1536, 512), F32R)
    w2 = din("w2", (1024, 512), F32R)
    w3 = din("w3", (1024, 512), F32R)
    bza_p = din("bza_p", (128, 2))
    b1_p = din("b1_p", (128, 4))
    b2_p = din("b2_p", (128, 4))
    b3_r = din("b3_r", (1, 512), F32R)
    ones_c = din("ones_c", (1, 128), F32R)
    oinv_m = din("oinv_m", (128, 1), F32R)
    out = nc.dram_tensor("out", [B_LOC, 512], F32, kind="ExternalOutput").ap()

    with tile.TileContext(nc) as tc:
        with (
            tc.tile_pool(name="wpool", bufs=1) as wp,
            tc.tile_pool(name="io", bufs=3) as io,
            tc.tile_pool(name="uv", bufs=1) as uv,
            tc.tile_pool(name="eph", bufs=2) as eph,
            tc.tile_pool(name="zcbp", bufs=1) as zcbp,
            tc.tile_pool(name="rows", bufs=1) as rows,
            tc.tile_pool(name="psz", bufs=2, space="PSUM") as psz,
            tc.tile_pool(name="psmu", bufs=1, space="PSUM") as psmu,
            tc.tile_pool(name="pssq", bufs=1, space="PSUM") as pssq,
            tc.tile_pool(name="psbc", bufs=2, space="PSUM") as psbc,
            tc.tile_pool(name="pso", bufs=2, space="PSUM") as pso,
        ):
            # ---- persistent weights / constants ----
            w1_sb = wp.tile([P, 12, 512], F32R)
            nc.sync.dma_start(w1_sb[:], w1.rearrange("(c p) o -> p c o", p=P))
            w2_sb = wp.tile([P, 8, 512], F32R)
            nc.sync.dma_start(w2_sb[:], w2.rearrange("(c p) o -> p c o", p=P))
            w3_sb = wp.tile([P, 8, 512], F32R)
            nc.sync.dma_start(w3_sb[:], w3.rearrange("(c p) o -> p c o", p=P))
            wza_sb = wp.tile([8, 2, 256], F32R)
            nc.sync.dma_start(wza_sb[:], wza[:, :, :])
            bza_sb = wp.tile([P, 2], F32)
            nc.sync.dma_start(bza_sb[:], bza_p[:, :])
            b1_sb = wp.tile([P, 4], F32)
            nc.sync.dma_start(b1_sb[:], b1_p[:, :])
            b2_sb = wp.tile([P, 4], F32)
            nc.sync.dma_start(b2_sb[:], b2_p[:, :])
            b3_sb = wp.tile([1, 512], F32R)
            nc.sync.dma_start(b3_sb[:], b3_r[:, :])
            ones_col = wp.tile([1, 128], F32R)
            nc.sync.dma_start(ones_col[:], ones_c[:, :])
            oinv_mcol = wp.tile([P, 1], F32R)   # 1/512 -> stats matmuls yield means
            nc.sync.dma_start(oinv_mcol[:], oinv_m[:, :])
            eps_sb = wp.tile([1, 1], F32)
            nc.vector.memset(eps_sb[:], LN_EPS)

            actT_sb = wp.tile([8, B_LOC], F32)
            nc.sync.dma_start(actT_sb[:], actT[:, :])

            def body():
                for b in range(NB):
                    bs = slice(b * BC, (b + 1) * BC)

                    # ==== l0: za = (pre-elu) APL(action) ====
                    u_act = eph.tile([8, BC], F32R, tag="u_act")
                    nc.gpsimd.tensor_scalar(u_act[:], actT_sb[:, bs],
                                            -1.0, 1.0, OP.max, OP.min)
                    v_act = eph.tile([8, BC], F32R, tag="v_act")
                    nc.gpsimd.tensor_scalar(v_act[:], actT_sb[:, bs],
                                            0.0, 1.0, OP.max, OP.min)
                    up_za = uv.tile([P, 2, BC], F32R, tag="up_za")
                    v_za = uv.tile([P, 2, BC], F32R, tag="v_za")
                    for o in range(2):
                        zps = psz.tile([P, BC], F32, tag="z")
                        nc.tensor.matmul(zps[:], wza_sb[:, 0, bass.ts(o, 128)],
                                         u_act[:], start=True, stop=False)
                        nc.tensor.matmul(zps[:], wza_sb[:, 1, bass.ts(o, 128)],
                                         v_act[:], start=False, stop=True)
                        zb = eph.tile([P, BC], F32, tag="zb_za")
                        nc.scalar.activation(zb[:], zps[:], AF.Identity,
                                             bias=bza_sb[:, o:o + 1])
                        nc.vector.tensor_scalar(v_za[:, o, :], zb[:], 0.0, 1.0,
                                                OP.max, OP.min)
                        nmin = eph.tile([P, BC], F32, tag="nmin_za")
                        nc.vector.tensor_scalar(nmin[:], zb[:], 0.0, None,
                                                OP.min)
                        ex = eph.tile([P, BC], F32, tag="ex_za")
                        nc.scalar.activation(ex[:], nmin[:], AF.Exp)
                        nc.vector.tensor_add(up_za[:, o, :], v_za[:, o, :],
                                             ex[:])

                    # ==== zs clips for l1 ====
                    u_zs = uv.tile([P, 4, BC], F32R, tag="u_zs")
                    v_zs = uv.tile([P, 4, BC], F32R, tag="v_zs")
                    for c in range(4):
                        zsraw = io.tile([P, BC], F32, tag="zsraw")
                        nc.sync.dma_start(zsraw[:], zsT[bass.ts(c, 128), bs])
                        nc.gpsimd.tensor_scalar(u_zs[:, c, :], zsraw[:],
                                                -1.0, 1.0, OP.max, OP.min)
                        nc.gpsimd.tensor_scalar(v_zs[:, c, :], zsraw[:],
                                                0.0, 1.0, OP.max, OP.min)

                    # ==== hidden layers l1, l2 ====
                    def hidden_layer(KC, w_sb, b_sb, rhs_fn, up_dst, v_dst):
                        zcbs = []
                        for o in range(4):
                            zps = psz.tile([P, BC], F32, tag="z")
                            for k in range(KC):
                                nc.tensor.matmul(zps[:],
                                                 w_sb[:, k, bass.ts(o, 128)],
                                                 rhs_fn(k),
                                                 start=(k == 0),
                                                 stop=(k == KC - 1))
                            zcb = zcbp.tile([P, BC], F32R, tag=f"zcb{o}")
                            nc.scalar.activation(zcb[:], zps[:], AF.Identity,
                                                 bias=b_sb[:, o:o + 1])
                            zcbs.append(zcb)
                        # stats: mean and mean-square via ones-matmuls (M=1)
                        mu_ps = psmu.tile([1, BC], F32, tag="mu")
                        for o in range(4):
                            nc.tensor.matmul(mu_ps[:], oinv_mcol[:],
                                             zcbs[o][:],
                                             start=(o == 0), stop=(o == 3))
                        sq_ps = pssq.tile([1, BC], F32, tag="sq")
                        for o in range(4):
                            zsq = eph.tile([P, BC], F32R, tag="zsq")
                            nc.scalar.activation(zsq[:], zcbs[o][:], AF.Square)
                            nc.tensor.matmul(sq_ps[:], oinv_mcol[:], zsq[:],
                                             start=(o == 0), stop=(o == 3))
                        negm = rows.tile([1, BC], F32R, tag="negm")
                        nc.vector.tensor_scalar(negm[:], mu_ps[:], -1.0, None,
                                                OP.mult)
                        m2 = rows.tile([1, BC], F32, tag="m2")
                        nc.scalar.activation(m2[:], negm[:], AF.Square)
                        var = rows.tile([1, BC], F32, tag="var")
                        nc.vector.scalar_tensor_tensor(var[:], sq_ps[:], 1.0,
                                                       m2[:], OP.mult,
                                                       OP.subtract)
                        std = rows.tile([1, BC], F32, tag="std")
                        nc.scalar.activation(std[:], var[:], AF.Sqrt,
                                             bias=eps_sb[:])
                        s_row = rows.tile([1, BC], F32R, tag="s_row")
                        with nc.allow_low_precision(
                                reason="rstd rounds to f32r for PE broadcast"):
                            nc.vector.reciprocal(s_row[:], std[:])
                        # broadcast -m and s across partitions (K=1 matmuls)
                        mb_ps = psbc.tile([P, BC], F32, tag="bc")
                        nc.tensor.matmul(mb_ps[:], ones_col[:], negm[:],
                                         start=True, stop=True)
                        sb_ps = psbc.tile([P, BC], F32, tag="bc")
                        nc.tensor.matmul(sb_ps[:], ones_col[:], s_row[:],
                                         start=True, stop=True)
                        for o in range(4):
                            tmp = eph.tile([P, BC], F32, tag="tmp")
                            nc.vector.tensor_add(tmp[:], zcbs[o][:], mb_ps[:])
                            h = eph.tile([P, BC], F32, tag="h")
                            nc.vector.tensor_mul(h[:], tmp[:], sb_ps[:])
                            nc.gpsimd.tensor_scalar(v_dst[:, o, :], h[:],
                                                    0.0, 1.0, OP.max, OP.min)
                            nmin = eph.tile([P, BC], F32, tag="nmin")
                            nc.gpsimd.tensor_scalar(nmin[:], h[:], 0.0, None,
                                                    OP.min)
                            ex = eph.tile([P, BC], F32, tag="ex")
                            nc.scalar.activation(ex[:], nmin[:], AF.Exp)
                            nc.vector.tensor_add(up_dst[:, o, :],
                                                 v_dst[:, o, :], ex[:])

                    def rhs1(k):
                        if k < 4:
                            return u_zs[:, k, :]
                        if k < 8:
                            return v_zs[:, k - 4, :]
                        if k < 10:
                            return up_za[:, k - 8, :]
                        return v_za[:, k - 10, :]

                    up1 = uv.tile([P, 4, BC], F32R, tag="up1")
                    v1 = uv.tile([P, 4, BC], F32R, tag="v1")
                    hidden_layer(12, w1_sb, b1_sb, rhs1, up1, v1)

                    def rhs2(k):
                        return up1[:, k, :] if k < 4 else v1[:, k - 4, :]

                    up2 = uv.tile([P, 4, BC], F32R, tag="up2")
                    v2 = uv.tile([P, 4, BC], F32R, tag="v2")
                    hidden_layer(8, w2_sb, b2_sb, rhs2, up2, v2)

                    # ==== l3: batch-major out ====
                    for q in range(4):
                        qs = bass.ts(q, 128)
                        ops = pso.tile([P, 512], F32, tag="o3")
                        for k in range(8):
                            lhsT = up2[:, k, qs] if k < 4 else v2[:, k - 4, qs]
                            nc.tensor.matmul(ops[:], lhsT, w3_sb[:, k, :],
                                             start=(k == 0), stop=False)
                        nc.tensor.matmul(ops[:], ones_col[:], b3_sb[:],
                                         start=False, stop=True)
                        osb = eph.tile([P, 512], F32, tag="osb")
                        nc.scalar.activation(osb[:], ops[:], AF.Copy)
                        nc.sync.dma_start(out[b * BC + q * 128:
                                              b * BC + (q + 1) * 128, :],
                                          osb[:])

            rep_ctx = (tc.For_i(0, n_reps, 1) if n_reps > 1
                       else contextlib.nullcontext())
            with rep_ctx:
                body()

    nc.compile()
    return nc


def fold_weights(W_za, W1, W2, W3):
    def fold(vals):
        V = vals.astype(np.float64)
        A = V[:, :, 1] - V[:, :, 0]
        C = V[:, :, 0] + V[:, :, 2] - 2.0 * V[:, :, 1]
        b = V[:, :, 1].sum(axis=0)
        return A, C, b

    A0, C0, b0 = fold(W_za)
    A1, C1, b1 = fold(W1)
    A2, C2, b2 = fold(W2)
    A3, C3, b3 = fold(W3)

    wza = np.stack([A0, C0], axis=1)                             # [8, 2, 256]
    w1 = np.concatenate([A1[:512], C1[:512], A1[512:], C1[512:]], axis=0)
    w2 = np.concatenate([A2, C2], axis=0)                        # [1024, 512]
    w3 = np.concatenate([A3, C3], axis=0)                        # [1024, 512]
    b1e = b1 - A1[512:].sum(axis=0)      # za u' carries +1 shift
    b2e = b2 - A2.sum(axis=0)
    b3e = b3 - A3.sum(axis=0)

    f = np.float32
    return {
        "wza": np.ascontiguousarray(wza, f),
        "w1": np.ascontiguousarray(w1, f),
        "w2": np.ascontiguousarray(w2, f),
        "w3": np.ascontiguousarray(w3, f),
        "bza_p": np.ascontiguousarray(b0.reshape(2, 128).T, f),
        "b1_p": np.ascontiguousarray(b1e.reshape(4, 128).T, f),
        "b2_p": np.ascontiguousarray(b2e.reshape(4, 128).T, f),
        "b3_r": np.ascontiguousarray(b3e.reshape(1, 512), f),
        "ones_c": np.ones((1, 128), f),
        "oinv_m": np.full((128, 1), 1.0 / 512.0, f),
    }


_NC_CACHE = {}


def get_module(n_reps=1):
    key = f"nc{n_reps}"
    if key not in _NC_CACHE:
        _NC_CACHE[key] = build_module(n_reps)
    return _NC_CACHE[key]


def make_in_maps(zs, action, W_za, W1, W2, W3):
    wmap = fold_weights(np.asarray(W_za), np.asarray(W1), np.asarray(W2),
                        np.asarray(W3))
    in_maps = []
    for c in range(NCORES):
        sl = slice(c * B_LOC, (c + 1) * B_LOC)
        m = dict(wmap)
        m["zsT"] = np.ascontiguousarray(np.asarray(zs)[sl].T, np.float32)
        m["actT"] = np.ascontiguousarray(np.asarray(action)[sl].T, np.float32)
        in_maps.append(m)
    return in_maps


def kernel(zs, action, W_za, W1, W2, W3, _trace=False, _tmpdir=None):
    nc = get_module()
    in_maps = make_in_maps(zs, action, W_za, W1, W2, W3)
    res = run_bass_kernel_spmd(nc, in_maps, core_ids=list(range(NCORES)),
                               trace=_trace, tmpdir=_tmpdir)
    out = np.concatenate([res.results[c]["out"] for c in range(NCORES)],
                         axis=0).astype(np.float32)
    if _trace:
        kernel.last_exec_time_ns = res.exec_time_ns
        kernel.last_results = res
    return out


# revision 26
# speedup vs baseline: 329.0813x; 3.1592x over previous
"""Trainium2 Bass kernel for nn_Encoder (KAN-style piecewise-linear MLP encoder).

Math: each adaptive piecewise-linear layer (P=3 knots on [-1,1]) collapses to
    out = u @ A + v @ C + bias,   u = clip(x,-1,1), v = clip(x,0,1)
with A = V1-V0, C = V0+V2-2*V1, bias = colsum(V1)  (hat basis sums to 1).
ELU never needs materializing: the next layer only consumes
    v' = clip(elu(h),0,1) = clip(h,0,1)
    u' = clip(elu(h),-1,1) + 1 = v' + exp(min(h,0))
and the +1 shift is folded into the next layer's bias (bias -= colsum(A_rows)).

Sharding: pure data-parallel, batch 16384 -> 8 x 2048. Activations are kept
feature-major ([feat, batch]) on chip so every matmul contracts over the
partition dim with no transposes; the host transposes the zs/action shards.
LayerNorm stats (feature = partition axis) are computed with ones-matmuls on
the PE and broadcast back with K=1 rank-1 matmuls. The last layer is computed
batch-major (activations stationary) so the output DMAs from SBUF in the
required [batch, 512] layout. Matmuls run as float32r (full PE rate).

n_reps>1 wraps the whole computation in a hardware For-loop; used only by the
local timing harness to measure per-iteration device time by wall-clock slope.
"""

import contextlib
import sys

sys.path.insert(0, "/opt/trn_rl_repo")

import numpy as np

import concourse.bass as bass  # noqa: E402
import concourse.tile as tile  # noqa: E402
from concourse import bacc, mybir  # noqa: E402
from concourse.bass_utils import run_bass_kernel_spmd  # noqa: E402

F32 = mybir.dt.float32
F32R = mybir.dt.float32r
AF = mybir.ActivationFunctionType
OP = mybir.AluOpType

NCORES = 8
B_LOC = 2048          # batch rows per core
BC = 512              # batch columns per chunk (psum free dim)
NB = B_LOC // BC      # 4 batch chunks
P = 128
LN_EPS = 1e-5


def build_module(n_reps=1):
    nc = bacc.Bacc("TRN2", target_bir_lowering=False, debug=False,
                   enable_asserts=False, num_devices=NCORES)

    def din(name, shape, dt=F32):
        return nc.dram_tensor(name, list(shape), dt, kind="ExternalInput").ap()

    zsT = din("zsT", (512, B_LOC))
    actT = din("actT", (8, B_LOC))
    wza = din("wza", (8, 2, 256), F32R)
    w1 = din("w1", (1536, 512), F32R)
    w2 = din("w2", (1024, 512), F32R)
    w3 = din("w3", (1024, 512), F32R)
    bza_p = din("bza_p", (128, 2))
    b1_p = din("b1_p", (128, 4))
    b2_p = din("b2_p", (128, 4))
    b3_r = din("b3_r", (1, 512), F32R)
    ones_c = din("ones_c", (1, 128), F32R)
    oinv_m = din("oinv_m", (128, 1), F32R)
    out = nc.dram_tensor("out", [B_LOC, 512], F32, kind="ExternalOutput").ap()

    with tile.TileContext(nc) as tc:
        with (
            tc.tile_pool(name="wpool", bufs=1) as wp,
            tc.tile_pool(name="io", bufs=3) as io,
            tc.tile_pool(name="uv", bufs=1) as uv,
            tc.tile_pool(name="eph", bufs=2) as eph,
            tc.tile_pool(name="zcbp", bufs=1) as zcbp,
            tc.tile_pool(name="rows", bufs=1) as rows,
            tc.tile_pool(name="psz", bufs=2, space="PSUM") as psz,
            tc.tile_pool(name="psmu", bufs=1, space="PSUM") as psmu,
            tc.tile_pool(name="pssq", bufs=1, space="PSUM") as pssq,
            tc.tile_pool(name="psbc", bufs=2, space="PSUM") as psbc,
            tc.tile_pool(name="pso", bufs=2, space="PSUM") as pso,
        ):
            # ---- persistent weights / constants ----
            w1_sb = wp.tile([P, 12, 512], F32R)
            nc.sync.dma_start(w1_sb[:], w1.rearrange("(c p) o -> p c o", p=P))
            w2_sb = wp.tile([P, 8, 512], F32R)
            nc.sync.dma_start(w2_sb[:], w2.rearrange("(c p) o -> p c o", p=P))
            w3_sb = wp.tile([P, 8, 512], F32R)
            nc.sync.dma_start(w3_sb[:], w3.rearrange("(c p) o -> p c o", p=P))
            wza_sb = wp.tile([8, 2, 256], F32R)
            nc.sync.dma_start(wza_sb[:], wza[:, :, :])
            bza_sb = wp.tile([P, 2], F32)
            nc.sync.dma_start(bza_sb[:], bza_p[:, :])
            b1_sb = wp.tile([P, 4], F32)
            nc.sync.dma_start(b1_sb[:], b1_p[:, :])
            b2_sb = wp.tile([P, 4], F32)
            nc.sync.dma_start(b2_sb[:], b2_p[:, :])
            b3_sb = wp.tile([1, 512], F32R)
            nc.sync.dma_start(b3_sb[:], b3_r[:, :])
            ones_col = wp.tile([1, 128], F32R)
            nc.sync.dma_start(ones_col[:], ones_c[:, :])
            oinv_mcol = wp.tile([P, 1], F32R)   # 1/512 -> stats matmuls yield means
            nc.sync.dma_start(oinv_mcol[:], oinv_m[:, :])
            eps_sb = wp.tile([1, 1], F32)
            nc.vector.memset(eps_sb[:], LN_EPS)

            actT_sb = wp.tile([8, B_LOC], F32)
            nc.sync.dma_start(actT_sb[:], actT[:, :])

            def body():
                for b in range(NB):
                    bs = slice(b * BC, (b + 1) * BC)

                    # ==== l0: za = (pre-elu) APL(action) ====
                    u_act = eph.tile([8, BC], F32R, tag="u_act")
                    nc.vector.tensor_scalar(u_act[:], actT_sb[:, bs],
                                            -1.0, 1.0, OP.max, OP.min)
                    v_act = eph.tile([8, BC], F32R, tag="v_act")
                    nc.vector.tensor_scalar(v_act[:], actT_sb[:, bs],
                                            0.0, 1.0, OP.max, OP.min)
                    up_za = uv.tile([P, 2, BC], F32R, tag="up_za")
                    v_za = uv.tile([P, 2, BC], F32R, tag="v_za")
                    for o in range(2):
                        zps = psz.tile([P, BC], F32, tag="z")
                        nc.tensor.matmul(zps[:], wza_sb[:, 0, bass.ts(o, 128)],
                                         u_act[:], start=True, stop=False)
                        nc.tensor.matmul(zps[:], wza_sb[:, 1, bass.ts(o, 128)],
                                         v_act[:], start=False, stop=True)
                        zb = eph.tile([P, BC], F32, tag="zb_za")
                        nc.scalar.activation(zb[:], zps[:], AF.Identity,
                                             bias=bza_sb[:, o:o + 1])
                        nc.vector.tensor_scalar(v_za[:, o, :], zb[:], 0.0, 1.0,
                                                OP.max, OP.min)
                        nmin = eph.tile([P, BC], F32, tag="nmin_za")
                        nc.vector.tensor_scalar(nmin[:], zb[:], 0.0, None,
                                                OP.min)
                        ex = eph.tile([P, BC], F32, tag="ex_za")
                        nc.scalar.activation(ex[:], nmin[:], AF.Exp)
                        nc.vector.tensor_add(up_za[:, o, :], v_za[:, o, :],
                                             ex[:])

                    # ==== zs clips for l1 ====
                    u_zs = uv.tile([P, 4, BC], F32R, tag="u_zs")
                    v_zs = uv.tile([P, 4, BC], F32R, tag="v_zs")
                    for c in range(4):
                        zsraw = io.tile([P, BC], F32, tag="zsraw")
                        nc.sync.dma_start(zsraw[:], zsT[bass.ts(c, 128), bs])
                        nc.vector.tensor_scalar(u_zs[:, c, :], zsraw[:],
                                                -1.0, 1.0, OP.max, OP.min)
                        nc.vector.tensor_scalar(v_zs[:, c, :], zsraw[:],
                                                0.0, 1.0, OP.max, OP.min)

                    # ==== hidden layers l1, l2 ====
                    def hidden_layer(KC, w_sb, b_sb, rhs_fn, up_dst, v_dst):
                        zcbs = []
                        for o in range(4):
                            zps = psz.tile([P, BC], F32, tag="z")
                            for k in range(KC):
                                nc.tensor.matmul(zps[:],
                                                 w_sb[:, k, bass.ts(o, 128)],
                                                 rhs_fn(k),
                                                 start=(k == 0),
                                                 stop=(k == KC - 1))
                            zcb = zcbp.tile([P, BC], F32R, tag=f"zcb{o}")
                            nc.scalar.activation(zcb[:], zps[:], AF.Identity,
                                                 bias=b_sb[:, o:o + 1])
                            zcbs.append(zcb)
                        # stats: mean and mean-square via ones-matmuls (M=1)
                        mu_ps = psmu.tile([1, BC], F32, tag="mu")
                        for o in range(4):
                            nc.tensor.matmul(mu_ps[:], oinv_mcol[:],
                                             zcbs[o][:],
                                             start=(o == 0), stop=(o == 3))
                        sq_ps = pssq.tile([1, BC], F32, tag="sq")
                        for o in range(4):
                            zsq = eph.tile([P, BC], F32R, tag="zsq")
                            nc.vector.tensor_mul(zsq[:], zcbs[o][:],
                                                 zcbs[o][:])
                            nc.tensor.matmul(sq_ps[:], oinv_mcol[:], zsq[:],
                                             start=(o == 0), stop=(o == 3))
                        negm = rows.tile([1, BC], F32R, tag="negm")
                        nc.vector.tensor_scalar(negm[:], mu_ps[:], -1.0, None,
                                                OP.mult)
                        m2 = rows.tile([1, BC], F32, tag="m2")
                        nc.vector.tensor_mul(m2[:], negm[:], negm[:])
                        var = rows.tile([1, BC], F32, tag="var")
                        nc.vector.scalar_tensor_tensor(var[:], sq_ps[:], 1.0,
                                                       m2[:], OP.mult,
                                                       OP.subtract)
                        std = rows.tile([1, BC], F32, tag="std")
                        nc.scalar.activation(std[:], var[:], AF.Sqrt,
                                             bias=eps_sb[:])
                        s_row = rows.tile([1, BC], F32R, tag="s_row")
                        with nc.allow_low_precision(
                                reason="rstd rounds to f32r for PE broadcast"):
                            nc.vector.reciprocal(s_row[:], std[:])
                        # broadcast -m and s across partitions (K=1 matmuls)
                        mb_ps = psbc.tile([P, BC], F32, tag="bc")
                        nc.tensor.matmul(mb_ps[:], ones_col[:], negm[:],
                                         start=True, stop=True)
                        sb_ps = psbc.tile([P, BC], F32, tag="bc")
                        nc.tensor.matmul(sb_ps[:], ones_col[:], s_row[:],
                                         start=True, stop=True)
                        for o in range(4):
                            tmp = eph.tile([P, BC], F32, tag="tmp")
                            nc.vector.tensor_add(tmp[:], zcbs[o][:], mb_ps[:])
                            h = eph.tile([P, BC], F32, tag="h")
                            nc.vector.tensor_mul(h[:], tmp[:], sb_ps[:])
                            nc.vector.tensor_scalar(v_dst[:, o, :], h[:],
                                                    0.0, 1.0, OP.max, OP.min)
                            nmin = eph.tile([P, BC], F32, tag="nmin")
                            nc.vector.tensor_scalar(nmin[:], h[:], 0.0, None,
                                                    OP.min)
                            ex = eph.tile([P, BC], F32, tag="ex")
                            nc.scalar.activation(ex[:], nmin[:], AF.Exp)
                            nc.vector.tensor_add(up_dst[:, o, :],
                                                 v_dst[:, o, :], ex[:])

                    def rhs1(k):
                        if k < 4:
                            return u_zs[:, k, :]
                        if k < 8:
                            return v_zs[:, k - 4, :]
                        if k < 10:
                            return up_za[:, k - 8, :]
                        return v_za[:, k - 10, :]

                    up1 = uv.tile([P, 4, BC], F32R, tag="up1")
                    v1 = uv.tile([P, 4, BC], F32R, tag="v1")
                    hidden_layer(12, w1_sb, b1_sb, rhs1, up1, v1)

                    def rhs2(k):
                        return up1[:, k, :] if k < 4 else v1[:, k - 4, :]

                    up2 = uv.tile([P, 4, BC], F32R, tag="up2")
                    v2 = uv.tile([P, 4, BC], F32R, tag="v2")
                    hidden_layer(8, w2_sb, b2_sb, rhs2, up2, v2)

                    # ==== l3: batch-major out ====
                    for q in range(4):
                        qs = bass.ts(q, 128)
                        ops = pso.tile([P, 512], F32, tag="o3")
                        for k in range(8):
                            lhsT = up2[:, k, qs] if k < 4 else v2[:, k - 4, qs]
                            nc.tensor.matmul(ops[:], lhsT, w3_sb[:, k, :],
                                             start=(k == 0), stop=False)
                        nc.tensor.matmul(ops[:], ones_col[:], b3_sb[:],
                                         start=False, stop=True)
                        osb = eph.tile([P, 512], F32, tag="osb")
                        nc.vector.tensor_copy(osb[:], ops[:])
                        nc.sync.dma_start(out[b * BC + q * 128:
                                              b * BC + (q + 1) * 128, :],
                                          osb[:])

            rep_ctx = (tc.For_i(0, n_reps, 1) if n_reps > 1
                       else contextlib.nullcontext())
            with rep_ctx:
                body()

    nc.compile()
    return nc


def fold_weights(W_za, W1, W2, W3):
    def fold(vals):
        V = vals.astype(np.float64)
        A = V[:, :, 1] - V[:, :, 0]
        C = V[:, :, 0] + V[:, :, 2] - 2.0 * V[:, :, 1]
        b = V[:, :, 1].sum(axis=0)
        return A, C, b

    A0, C0, b0 = fold(W_za)
    A1, C1, b1 = fold(W1)
    A2, C2, b2 = fold(W2)
    A3, C3, b3 = fold(W3)

    wza = np.stack([A0, C0], axis=1)                             # [8, 2, 256]
    w1 = np.concatenate([A1[:512], C1[:512], A1[512:], C1[512:]], axis=0)
    w2 = np.concatenate([A2, C2], axis=0)                        # [1024, 512]
    w3 = np.concatenate([A3, C3], axis=0)                        # [1024, 512]
    b1e = b1 - A1[512:].sum(axis=0)      # za u' carries +1 shift
    b2e = b2 - A2.sum(axis=0)
    b3e = b3 - A3.sum(axis=0)

    f = np.float32
    return {
        "wza": np.ascontiguousarray(wza, f),
        "w1": np.ascontiguousarray(w1, f),
        "w2": np.ascontiguousarray(w2, f),
        "w3": np.ascontiguousarray(w3, f),
        "bza_p": np.ascontiguousarray(b0.reshape(2, 128).T, f),
        "b1_p": np.ascontiguousarray(b1e.reshape(4, 128).T, f),
        "b2_p": np.ascontiguousarray(b2e.reshape(4, 128).T, f),
        "b3_r": np.ascontiguousarray(b3e.reshape(1, 512), f),
        "ones_c": np.ones((1, 128), f),
        "oinv_m": np.full((128, 1), 1.0 / 512.0, f),
    }


_NC_CACHE = {}


def get_module(n_reps=1):
    key = f"nc{n_reps}"
    if key not in _NC_CACHE:
        _NC_CACHE[key] = build_module(n_reps)
    return _NC_CACHE[key]


def make_in_maps(zs, action, W_za, W1, W2, W3):
    wmap = fold_weights(np.asarray(W_za), np.asarray(W1), np.asarray(W2),
                        np.asarray(W3))
    in_maps = []
    for c in range(NCORES):
        sl = slice(c * B_LOC, (c + 1) * B_LOC)
        m = dict(wmap)
        m["zsT"] = np.ascontiguousarray(np.asarray(zs)[sl].T, np.float32)
        m["actT"] = np.ascontiguousarray(np.asarray(action)[sl].T, np.float32)
        in_maps.append(m)
    return in_maps


def kernel(zs, action, W_za, W1, W2, W3, _trace=False, _tmpdir=None):
    nc = get_module()
    in_maps = make_in_maps(zs, action, W_za, W1, W2, W3)
    res = run_bass_kernel_spmd(nc, in_maps, core_ids=list(range(NCORES)),
                               trace=_trace, tmpdir=_tmpdir)
    out = np.concatenate([res.results[c]["out"] for c in range(NCORES)],
                         axis=0).astype(np.float32)
    if _trace:
        kernel.last_exec_time_ns = res.exec_time_ns
        kernel.last_results = res
    return out


# revision 28
# speedup vs baseline: 334.2277x; 1.0156x over previous
"""Trainium2 Bass kernel for nn_Encoder (KAN-style piecewise-linear MLP encoder).

Math: each adaptive piecewise-linear layer (P=3 knots on [-1,1]) collapses to
    out = u @ A + v @ C + bias,   u = clip(x,-1,1), v = clip(x,0,1)
with A = V1-V0, C = V0+V2-2*V1, bias = colsum(V1)  (hat basis sums to 1).
ELU never needs materializing: the next layer only consumes
    v' = clip(elu(h),0,1) = clip(h,0,1)
    u' = clip(elu(h),-1,1) + 1 = v' + exp(min(h,0))
and the +1 shift is folded into the next layer's bias (bias -= colsum(A_rows)).

Sharding: pure data-parallel, batch 16384 -> 8 x 2048. Activations are kept
feature-major ([feat, batch]) on chip so every matmul contracts over the
partition dim with no transposes; the host transposes the zs/action shards.
LayerNorm stats (feature = partition axis) are computed with ones-matmuls on
the PE and broadcast back with K=1 rank-1 matmuls. The last layer is computed
batch-major (activations stationary) so the output DMAs from SBUF in the
required [batch, 512] layout. Matmuls run as float32r (full PE rate).

n_reps>1 wraps the whole computation in a hardware For-loop; used only by the
local timing harness to measure per-iteration device time by wall-clock slope.
"""

import contextlib
import sys

sys.path.insert(0, "/opt/trn_rl_repo")

import numpy as np

import concourse.bass as bass  # noqa: E402
import concourse.tile as tile  # noqa: E402
from concourse import bacc, mybir  # noqa: E402
from concourse.bass_utils import run_bass_kernel_spmd  # noqa: E402

F32 = mybir.dt.float32
F32R = mybir.dt.float32r
AF = mybir.ActivationFunctionType
OP = mybir.AluOpType

NCORES = 8
B_LOC = 2048          # batch rows per core
BC = 512              # batch columns per chunk (psum free dim)
NB = B_LOC // BC      # 4 batch chunks
P = 128
LN_EPS = 1e-5


def build_module(n_reps=1):
    nc = bacc.Bacc("TRN2", target_bir_lowering=False, debug=False,
                   enable_asserts=False, num_devices=NCORES)

    def din(name, shape, dt=F32):
        return nc.dram_tensor(name, list(shape), dt, kind="ExternalInput").ap()

    zsT = din("zsT", (512, B_LOC))
    actT = din("actT", (8, B_LOC))
    wza = din("wza", (8, 2, 256), F32R)
    w1 = din("w1", (1536, 512), F32R)
    w2 = din("w2", (1024, 512), F32R)
    w3 = din("w3", (1024, 512), F32R)
    bza_p = din("bza_p", (128, 2))
    b1_p = din("b1_p", (128, 4))
    b2_p = din("b2_p", (128, 4))
    b3_bc = din("b3_bc", (128, 512))
    ones_c = din("ones_c", (1, 128), F32R)
    oinv_m = din("oinv_m", (128, 1), F32R)
    out = nc.dram_tensor("out", [B_LOC, 512], F32, kind="ExternalOutput").ap()

    with tile.TileContext(nc) as tc:
        with (
            tc.tile_pool(name="wpool", bufs=1) as wp,
            tc.tile_pool(name="io", bufs=3) as io,
            tc.tile_pool(name="uv", bufs=1) as uv,
            tc.tile_pool(name="eph", bufs=2) as eph,
            tc.tile_pool(name="zcbp", bufs=1) as zcbp,
            tc.tile_pool(name="rows", bufs=1) as rows,
            tc.tile_pool(name="psz", bufs=3, space="PSUM") as psz,
            tc.tile_pool(name="psmu", bufs=1, space="PSUM") as psmu,
            tc.tile_pool(name="pssq", bufs=1, space="PSUM") as pssq,
            tc.tile_pool(name="psbc", bufs=2, space="PSUM") as psbc,
            tc.tile_pool(name="pso", bufs=1, space="PSUM") as pso,
        ):
            # ---- persistent weights / constants ----
            w1_sb = wp.tile([P, 12, 512], F32R)
            nc.sync.dma_start(w1_sb[:], w1.rearrange("(c p) o -> p c o", p=P))
            w2_sb = wp.tile([P, 8, 512], F32R)
            nc.sync.dma_start(w2_sb[:], w2.rearrange("(c p) o -> p c o", p=P))
            w3_sb = wp.tile([P, 8, 512], F32R)
            nc.sync.dma_start(w3_sb[:], w3.rearrange("(c p) o -> p c o", p=P))
            wza_sb = wp.tile([8, 2, 256], F32R)
            nc.sync.dma_start(wza_sb[:], wza[:, :, :])
            bza_sb = wp.tile([P, 2], F32)
            nc.sync.dma_start(bza_sb[:], bza_p[:, :])
            b1_sb = wp.tile([P, 4], F32)
            nc.sync.dma_start(b1_sb[:], b1_p[:, :])
            b2_sb = wp.tile([P, 4], F32)
            nc.sync.dma_start(b2_sb[:], b2_p[:, :])
            b3_sb = wp.tile([P, 512], F32)
            nc.sync.dma_start(b3_sb[:], b3_bc[:, :])
            ones_col = wp.tile([1, 128], F32R)
            nc.sync.dma_start(ones_col[:], ones_c[:, :])
            oinv_mcol = wp.tile([P, 1], F32R)   # 1/512 -> stats matmuls yield means
            nc.sync.dma_start(oinv_mcol[:], oinv_m[:, :])
            eps_sb = wp.tile([1, 1], F32)
            nc.vector.memset(eps_sb[:], LN_EPS)

            actT_sb = wp.tile([8, B_LOC], F32)
            nc.sync.dma_start(actT_sb[:], actT[:, :])

            def body():
                for b in range(NB):
                    bs = slice(b * BC, (b + 1) * BC)

                    # ==== l0: za = (pre-elu) APL(action) ====
                    u_act = eph.tile([8, BC], F32R, tag="u_act")
                    nc.vector.tensor_scalar(u_act[:], actT_sb[:, bs],
                                            -1.0, 1.0, OP.max, OP.min)
                    v_act = eph.tile([8, BC], F32R, tag="v_act")
                    nc.vector.tensor_scalar(v_act[:], actT_sb[:, bs],
                                            0.0, 1.0, OP.max, OP.min)
                    up_za = uv.tile([P, 2, BC], F32R, tag="up_za")
                    v_za = uv.tile([P, 2, BC], F32R, tag="v_za")
                    for o in range(2):
                        zps = psz.tile([P, BC], F32, tag="z")
                        nc.tensor.matmul(zps[:], wza_sb[:, 0, bass.ts(o, 128)],
                                         u_act[:], start=True, stop=False)
                        nc.tensor.matmul(zps[:], wza_sb[:, 1, bass.ts(o, 128)],
                                         v_act[:], start=False, stop=True)
                        zb = eph.tile([P, BC], F32, tag="zb_za")
                        nc.scalar.activation(zb[:], zps[:], AF.Identity,
                                             bias=bza_sb[:, o:o + 1])
                        nc.vector.tensor_scalar(v_za[:, o, :], zb[:], 0.0, 1.0,
                                                OP.max, OP.min)
                        nmin = eph.tile([P, BC], F32, tag="nmin_za")
                        nc.vector.tensor_scalar(nmin[:], zb[:], 0.0, None,
                                                OP.min)
                        ex = eph.tile([P, BC], F32, tag="ex_za")
                        nc.scalar.activation(ex[:], nmin[:], AF.Exp)
                        nc.vector.tensor_add(up_za[:, o, :], v_za[:, o, :],
                                             ex[:])

                    # ==== zs clips for l1 ====
                    u_zs = uv.tile([P, 4, BC], F32R, tag="u_zs")
                    v_zs = uv.tile([P, 4, BC], F32R, tag="v_zs")
                    for c in range(4):
                        zsraw = io.tile([P, BC], F32, tag="zsraw")
                        nc.sync.dma_start(zsraw[:], zsT[bass.ts(c, 128), bs])
                        nc.vector.tensor_scalar(u_zs[:, c, :], zsraw[:],
                                                -1.0, 1.0, OP.max, OP.min)
                        nc.vector.tensor_scalar(v_zs[:, c, :], zsraw[:],
                                                0.0, 1.0, OP.max, OP.min)

                    # ==== hidden layers l1, l2 ====
                    def hidden_layer(KC, w_sb, b_sb, rhs_fn, up_dst, v_dst):
                        zcbs = []
                        for o in range(4):
                            zps = psz.tile([P, BC], F32, tag="z")
                            for k in range(KC):
                                nc.tensor.matmul(zps[:],
                                                 w_sb[:, k, bass.ts(o, 128)],
                                                 rhs_fn(k),
                                                 start=(k == 0),
                                                 stop=(k == KC - 1))
                            zcb = zcbp.tile([P, BC], F32R, tag=f"zcb{o}")
                            nc.scalar.activation(zcb[:], zps[:], AF.Identity,
                                                 bias=b_sb[:, o:o + 1])
                            zcbs.append(zcb)
                        # stats: mean and mean-square via ones-matmuls (M=1)
                        mu_ps = psmu.tile([1, BC], F32, tag="mu")
                        for o in range(4):
                            nc.tensor.matmul(mu_ps[:], oinv_mcol[:],
                                             zcbs[o][:],
                                             start=(o == 0), stop=(o == 3))
                        sq_ps = pssq.tile([1, BC], F32, tag="sq")
                        for o in range(4):
                            zsq = eph.tile([P, BC], F32R, tag="zsq")
                            nc.vector.tensor_mul(zsq[:], zcbs[o][:],
                                                 zcbs[o][:])
                            nc.tensor.matmul(sq_ps[:], oinv_mcol[:], zsq[:],
                                             start=(o == 0), stop=(o == 3))
                        negm = rows.tile([1, BC], F32R, tag="negm")
                        nc.vector.tensor_scalar(negm[:], mu_ps[:], -1.0, None,
                                                OP.mult)
                        m2 = rows.tile([1, BC], F32, tag="m2")
                        nc.vector.tensor_mul(m2[:], negm[:], negm[:])
                        var = rows.tile([1, BC], F32, tag="var")
                        nc.vector.scalar_tensor_tensor(var[:], sq_ps[:], 1.0,
                                                       m2[:], OP.mult,
                                                       OP.subtract)
                        std = rows.tile([1, BC], F32, tag="std")
                        nc.scalar.activation(std[:], var[:], AF.Sqrt,
                                             bias=eps_sb[:])
                        s_row = rows.tile([1, BC], F32R, tag="s_row")
                        with nc.allow_low_precision(
                                reason="rstd rounds to f32r for PE broadcast"):
                            nc.vector.reciprocal(s_row[:], std[:])
                        # broadcast -m and s across partitions (K=1 matmuls)
                        mb_ps = psbc.tile([P, BC], F32, tag="bc")
                        nc.tensor.matmul(mb_ps[:], ones_col[:], negm[:],
                                         start=True, stop=True)
                        sb_ps = psbc.tile([P, BC], F32, tag="bc")
                        nc.tensor.matmul(sb_ps[:], ones_col[:], s_row[:],
                                         start=True, stop=True)
                        for o in range(4):
                            tmp = eph.tile([P, BC], F32, tag="tmp")
                            nc.vector.tensor_add(tmp[:], zcbs[o][:], mb_ps[:])
                            h = eph.tile([P, BC], F32, tag="h")
                            nc.vector.tensor_mul(h[:], tmp[:], sb_ps[:])
                            nc.vector.tensor_scalar(v_dst[:, o, :], h[:],
                                                    0.0, 1.0, OP.max, OP.min)
                            nmin = eph.tile([P, BC], F32, tag="nmin")
                            nc.vector.tensor_scalar(nmin[:], h[:], 0.0, None,
                                                    OP.min)
                            ex = eph.tile([P, BC], F32, tag="ex")
                            nc.scalar.activation(ex[:], nmin[:], AF.Exp)
                            nc.vector.tensor_add(up_dst[:, o, :],
                                                 v_dst[:, o, :], ex[:])

                    def rhs1(k):
                        if k < 4:
                            return u_zs[:, k, :]
                        if k < 8:
                            return v_zs[:, k - 4, :]
                        if k < 10:
                            return up_za[:, k - 8, :]
                        return v_za[:, k - 10, :]

                    up1 = uv.tile([P, 4, BC], F32R, tag="up1")
                    v1 = uv.tile([P, 4, BC], F32R, tag="v1")
                    hidden_layer(12, w1_sb, b1_sb, rhs1, up1, v1)

                    def rhs2(k):
                        return up1[:, k, :] if k < 4 else v1[:, k - 4, :]

                    up2 = uv.tile([P, 4, BC], F32R, tag="up2")
                    v2 = uv.tile([P, 4, BC], F32R, tag="v2")
                    hidden_layer(8, w2_sb, b2_sb, rhs2, up2, v2)

                    # ==== l3: batch-major out ====
                    for q in range(4):
                        qs = bass.ts(q, 128)
                        ops = pso.tile([P, 512], F32, tag="o3")
                        for k in range(8):
                            lhsT = up2[:, k, qs] if k < 4 else v2[:, k - 4, qs]
                            nc.tensor.matmul(ops[:], lhsT, w3_sb[:, k, :],
                                             start=(k == 0), stop=(k == 7))
                        osb = eph.tile([P, 512], F32, tag="osb")
                        nc.vector.scalar_tensor_tensor(osb[:], ops[:], 1.0,
                                                       b3_sb[:], OP.mult,
                                                       OP.add)
                        nc.sync.dma_start(out[b * BC + q * 128:
                                              b * BC + (q + 1) * 128, :],
                                          osb[:])

            rep_ctx = (tc.For_i(0, n_reps, 1) if n_reps > 1
                       else contextlib.nullcontext())
            with rep_ctx:
                body()

    nc.compile()
    return nc


def fold_weights(W_za, W1, W2, W3):
    def fold(vals):
        V = vals.astype(np.float64)
        A = V[:, :, 1] - V[:, :, 0]
        C = V[:, :, 0] + V[:, :, 2] - 2.0 * V[:, :, 1]
        b = V[:, :, 1].sum(axis=0)
        return A, C, b

    A0, C0, b0 = fold(W_za)
    A1, C1, b1 = fold(W1)
    A2, C2, b2 = fold(W2)
    A3, C3, b3 = fold(W3)

    wza = np.stack([A0, C0], axis=1)                             # [8, 2, 256]
    w1 = np.concatenate([A1[:512], C1[:512], A1[512:], C1[512:]], axis=0)
    w2 = np.concatenate([A2, C2], axis=0)                        # [1024, 512]
    w3 = np.concatenate([A3, C3], axis=0)                        # [1024, 512]
    b1e = b1 - A1[512:].sum(axis=0)      # za u' carries +1 shift
    b2e = b2 - A2.sum(axis=0)
    b3e = b3 - A3.sum(axis=0)

    f = np.float32
    return {
        "wza": np.ascontiguousarray(wza, f),
        "w1": np.ascontiguousarray(w1, f),
        "w2": np.ascontiguousarray(w2, f),
        "w3": np.ascontiguousarray(w3, f),
        "bza_p": np.ascontiguousarray(b0.reshape(2, 128).T, f),
        "b1_p": np.ascontiguousarray(b1e.reshape(4, 128).T, f),
        "b2_p": np.ascontiguousarray(b2e.reshape(4, 128).T, f),
        "b3_bc": np.ascontiguousarray(np.broadcast_to(b3e, (128, 512)), f),
        "ones_c": np.ones((1, 128), f),
        "oinv_m": np.full((128, 1), 1.0 / 512.0, f),
    }


_NC_CACHE = {}


def get_module(n_reps=1):
    key = f"nc{n_reps}"
    if key not in _NC_CACHE:
        _NC_CACHE[key] = build_module(n_reps)
    return _NC_CACHE[key]


def make_in_maps(zs, action, W_za, W1, W2, W3):
    wmap = fold_weights(np.asarray(W_za), np.asarray(W1), np.asarray(W2),
                        np.asarray(W3))
    in_maps = []
    for c in range(NCORES):
        sl = slice(c * B_LOC, (c + 1) * B_LOC)
        m = dict(wmap)
        m["zsT"] = np.ascontiguousarray(np.asarray(zs)[sl].T, np.float32)
        m["actT"] = np.ascontiguousarray(np.asarray(action)[sl].T, np.float32)
        in_maps.append(m)
    return in_maps


def kernel(zs, action, W_za, W1, W2, W3, _trace=False, _tmpdir=None):
    nc = get_module()
    in_maps = make_in_maps(zs, action, W_za, W1, W2, W3)
    res = run_bass_kernel_spmd(nc, in_maps, core_ids=list(range(NCORES)),
                               trace=_trace, tmpdir=_tmpdir)
    out = np.concatenate([res.results[c]["out"] for c in range(NCORES)],
                         axis=0).astype(np.float32)
    if _trace:
        kernel.last_exec_time_ns = res.exec_time_ns
        kernel.last_results = res
    return out
